# revision 38
# baseline (speedup 1.0000x reference)
"""GCN dialogue manager kernel for 8 trn2 NeuronCores.

Math (reference):
    h   = x @ W_gcn
    deg = in_deg(dst) + 1   (self loops)
    dinv = rsqrt(deg)
    agg[d] = sum_{e:(s->d)} dinv[s]*dinv[d]*h[s] + dinv[d]^2*h[d] + b_gcn
    out = agg @ W_act + b_act

Kernel strategy (dst-sharded, edges partitioned by destination):
    out[d] = dinv[d] * (sum_{slots of d} x~[s]) @ (W_gcn@W_act) + (b_gcn@W_act + b_act)
  with x~[s] = dinv[s]*x[s] prescaled on the host and stored as fp16
  half-tables (256B rows halve gather traffic vs fp32; int16 gather idx
  limit forces the two-table split). Self loops are ordinary slots.
  - 8 cores each own 6250 destination nodes (49 blocks of 128).
  - Edges (incl. self loops) are bucketed by (core, dst-block, src-half) on
    the host into a static per-group schedule; each group gathers exactly
    maxcnt[g] = cross-core max real count (16-quantized) slots, so no
    full-tile zero-row padding is transferred.
  - On device: dma_gather fetches x~ rows per slot (4 SWDGE queues,
    enlarged dynamic-DMA scratch ring to decouple Q7 descriptor gen from
    SDMA drain).
  - A fp16 one-hot matrix sel[slot, dst] = (dstloc[slot]==iota) routes each
    slot to its dst row via TensorE matmuls accumulated transposed in fp32
    PSUM: accT[feat, dst] += xg[slot, feat]^T @ sel[slot, dst]. Per block:
    outp[dst, A] = accT^T @ (W_gcn@W_act), scaled by dinv[dst] (ACT
    per-partition scale), plus broadcast bias, written out.
"""

import os
import sys

for _p in ("/opt/trn_rl_repo",):
    if _p not in sys.path and os.path.isdir(_p):
        sys.path.insert(0, _p)

import numpy as np

# ---- problem constants (hardcoded per spec) ----
N, E, F, HID, A = 50000, 600000, 128, 128, 64
P = 128                      # partitions
NCORE = 8
DST_PER_CORE = 6250          # N / 8
NBLK = 49                    # ceil(6250/128) dst blocks per core
OUT_ROWS = NBLK * P          # 6272 padded out rows per core
HALF = 25152                 # nodes [0,HALF) in table A, [HALF,...) in table B
XROWS = 25216                # rows per half table (HALF + 64 zero pad rows)
ZROW_A = 25152               # a zero row in table A (explicit pad row)
ZROW_B = 25024               # node 50176 -> xb row 25024 (zero: node >= N)
MAXCNT_CAP = 1024            # hard cap: 1024 slots per gather call (ring cap)
CHUNK = 2                    # dst blocks per compute chunk
_CHUNKS = [(c * CHUNK, min(CHUNK, NBLK - c * CHUNK)) for c in range((NBLK + CHUNK - 1) // CHUNK)]

_prog_cache = {}


def _build_program(maxcnt):
    """Build the Bass program shared by all 8 cores.

    maxcnt: tuple of 98 ints — exact slots gathered per (block, half) group
    (cross-core max real count, 16-quantized), group g = blk*2 + half."""
    key = tuple(maxcnt)
    if key in _prog_cache:
        return _prog_cache[key]

    import concourse.bacc as bacc
    import concourse.mybir as mybir
    import concourse.tile as tile
    from concourse.masks import make_identity

    f32 = mybir.dt.float32
    f16 = mybir.dt.float16
    i32 = mybir.dt.int32
    i16 = mybir.dt.int16
    Alu = mybir.AluOpType
    Act = mybir.ActivationFunctionType

    ntile = [(c + P - 1) // P for c in maxcnt]
    col_start = np.concatenate([[0], np.cumsum(ntile)]).astype(int)
    tot_col = int(col_start[-1])
    tot_slot = tot_col * P

    nc = bacc.Bacc(None, num_swdge_queues=4, dynamic_dma_scratch_size=49152)

    xa = nc.dram_tensor("xa", [XROWS, F], f16, kind="ExternalInput")
    xb = nc.dram_tensor("xb", [XROWS, F], f16, kind="ExternalInput")
    idxs = nc.dram_tensor("idxs", [P, tot_slot // 16], i16, kind="ExternalInput")
    dstloc = nc.dram_tensor("dstloc", [P, tot_col], f16, kind="ExternalInput")
    xown = nc.dram_tensor("xown", [P, NBLK * F], f16, kind="ExternalInput")
    dinvd = nc.dram_tensor("dinvd", [P, NBLK], f32, kind="ExternalInput")
    wgT = nc.dram_tensor("wgT", [HID, F], f32, kind="ExternalInput")
    wact = nc.dram_tensor("wact", [HID, A], f32, kind="ExternalInput")
    bgcn = nc.dram_tensor("bgcn", [HID, 1], f32, kind="ExternalInput")
    bact = nc.dram_tensor("bact", [1, A], f32, kind="ExternalInput")
    out = nc.dram_tensor("out", [OUT_ROWS, A], f32, kind="ExternalOutput")

    with tile.TileContext(nc) as tc:
        with (
            tc.tile_pool(name="const", bufs=1) as cpool,
            tc.tile_pool(name="cpsum", bufs=1, space="PSUM") as cpsum,
            tc.tile_pool(name="xg", bufs=4) as xgpool,
            tc.tile_pool(name="sel", bufs=2) as spool,
            tc.tile_pool(name="acc", bufs=4, space="PSUM") as accpool,
            tc.tile_pool(name="outp", bufs=3, space="PSUM") as outppool,
            tc.tile_pool(name="flush", bufs=6) as fpool,
        ):
            # ---- constants / prologue ----
            # idx table loaded in per-chunk slices so the first gathers
            # don't wait on the whole transfer
            idx_sb = cpool.tile([P, tot_slot // 16], i16)
            for (b0, ncb) in _CHUNKS:
                a = int(col_start[b0 * 2]) * P // 16
                z = int(col_start[(b0 + ncb) * 2]) * P // 16
                nc.sync.dma_start(out=idx_sb[:, a:z], in_=idxs[:, a:z])
            dstloc_sb = cpool.tile([P, tot_col], f16)
            nc.sync.dma_start(out=dstloc_sb[:], in_=dstloc[:])

            dinvdst = cpool.tile([P, NBLK], f32)
            nc.sync.dma_start(out=dinvdst[:], in_=dinvd[:])

            iota_i = cpool.tile([P, P], i32)
            nc.gpsimd.iota(iota_i[:], pattern=[[1, P]], base=0, channel_multiplier=0)
            iota_f = cpool.tile([P, P], f16)
            nc.vector.tensor_copy(out=iota_f[:], in_=iota_i[:])

            # own-dst x~ slab (self-loop term) + fp16 identity for its matmul
            xown_sb = cpool.tile([P, NBLK * F], f16)
            nc.sync.dma_start(out=xown_sb[:], in_=xown[:])
            ident_f = cpool.tile([P, P], f32)
            make_identity(nc, ident_f[:])
            ident16 = cpool.tile([P, P], f16)
            nc.vector.tensor_copy(out=ident16[:], in_=ident_f[:])

            wgT_sb = cpool.tile([HID, F], f32)
            nc.sync.dma_start(out=wgT_sb[:], in_=wgT[:])
            wact_sb = cpool.tile([HID, A], f32)
            nc.sync.dma_start(out=wact_sb[:], in_=wact[:])
            wf_ps = cpsum.tile([F, A], f32, space="PSUM", tag="cps")
            nc.tensor.matmul(wf_ps[:], lhsT=wgT_sb[:], rhs=wact_sb[:], start=True, stop=True)
            wf_sb = cpool.tile([F, A], f32)
            nc.vector.tensor_copy(out=wf_sb[:], in_=wf_ps[:])

            bgcn_sb = cpool.tile([HID, 1], f32)
            nc.sync.dma_start(out=bgcn_sb[:], in_=bgcn[:])
            bact_sb = cpool.tile([1, A], f32)
            nc.sync.dma_start(out=bact_sb[:], in_=bact[:])
            cb_ps = cpsum.tile([1, A], f32, space="PSUM", tag="cps")
            nc.tensor.matmul(cb_ps[:], lhsT=bgcn_sb[:], rhs=wact_sb[:], start=True, stop=True)
            cb_sb = cpool.tile([1, A], f32)
            nc.vector.tensor_copy(out=cb_sb[:], in_=cb_ps[:])
            nc.vector.tensor_tensor(out=cb_sb[:], in0=cb_sb[:], in1=bact_sb[:], op=Alu.add)
            bias_full = cpool.tile([P, A], f32)
            nc.gpsimd.partition_broadcast(bias_full[:], cb_sb[:])

            num_regs = {v: nc.gpsimd.to_reg(v)
                        for v in sorted(set(int(c) for c in maxcnt))}

            # prime the gather buffers: the tail of each group's last tile is
            # never written by the gather; sel is 0 there, but stale SBUF bits
            # could be NaN and 0*NaN would poison the accumulation.
            max_ncols = max(
                int(col_start[(b0 + ncb) * 2] - col_start[b0 * 2])
                for (b0, ncb) in _CHUNKS
            )
            for _ in range(4):
                zt = xgpool.tile([P, max_ncols, F], f16, tag="xg")
                nc.vector.memset(zt[:], 0.0)

            # ---- main loop over chunks of dst blocks ----
            qn = 0
            for ci, (b0, ncb) in enumerate(_CHUNKS):
                c0 = int(col_start[b0 * 2])
                ncols = int(col_start[(b0 + ncb) * 2] - c0)
                xg = xgpool.tile([P, max_ncols, F], f16, tag="xg")
                for i in range(ncb):
                    for h, tab in ((0, xa), (1, xb)):
                        g = (b0 + i) * 2 + h
                        num = int(maxcnt[g])
                        nt = int(ntile[g])
                        crel = int(col_start[g]) - c0
                        s0 = int(col_start[g]) * P
                        nc.gpsimd.dma_gather(
                            xg[:, crel: crel + nt, :],
                            tab[:],
                            idx_sb[:, s0 // 16: s0 // 16 + (num + 15) // 16],
                            num,
                            num_regs[num],
                            F,
                            queue_num=qn % 4,
                        )
                        qn += 1
                qn += 1  # rotate the queue each chunk starts on
                # fp16 one-hot selection (broadcast TT on DVE)
                sel = spool.tile([P, ncols, P], f16, tag="sel")
                nc.vector.tensor_tensor(
                    out=sel[:],
                    in0=dstloc_sb[:, c0:c0 + ncols].unsqueeze(2).broadcast_to([P, ncols, P]),
                    in1=iota_f[:].unsqueeze(1).broadcast_to([P, ncols, P]),
                    op=Alu.is_equal,
                )
                for i in range(ncb):
                    b = b0 + i
                    gA, gB = b * 2, b * 2 + 1
                    accT = accpool.tile([P, P], f32, space="PSUM", tag="acc")
                    # self-loop term: accT[feat, dst] += xown_b[dst, feat]^T
                    nc.tensor.matmul(
                        accT[:],
                        lhsT=xown_sb[:, b * F:(b + 1) * F],
                        rhs=ident16[:],
                        start=True,
                        stop=False,
                    )
                    cols = list(range(int(col_start[gA]) - c0, int(col_start[gB + 1]) - c0))
                    for j, col in enumerate(cols):
                        nc.tensor.matmul(
                            accT[:],
                            lhsT=xg[:, col, :],
                            rhs=sel[:, col, :],
                            start=False,
                            stop=(j == len(cols) - 1),
                        )
                    # flush block b: accT[feat, dst] -> out rows
                    accS = fpool.tile([P, P], f32, tag="accS")
                    nc.scalar.activation(accS[:], accT[:], Act.Copy)
                    outp = outppool.tile([P, A], f32, space="PSUM", tag="outp")
                    nc.tensor.matmul(outp[:], lhsT=accS[:], rhs=wf_sb[:], start=True, stop=True)
                    out_sb = fpool.tile([P, A], f32, tag="outs")
                    nc.scalar.activation(out_sb[:], outp[:], Act.Copy, scale=dinvdst[:, b:b + 1])
                    nc.vector.tensor_tensor(out=out_sb[:], in0=out_sb[:], in1=bias_full[:], op=Alu.add)
                    nc.sync.dma_start(out=out[b * P:(b + 1) * P, :], in_=out_sb[:])

    nc.compile()
    _prog_cache[key] = nc
    return nc


def _preprocess(x, edge_index):
    """Host-side sharding: bucket edges by (core, dst block, src half), build
    the per-slot gather index/dst-position arrays, and prescale x rows by
    dinv[src] into fp16 half tables."""
    src = np.asarray(edge_index[0], dtype=np.int64)
    dst = np.asarray(edge_index[1], dtype=np.int64)

    in_deg = np.bincount(dst, minlength=N).astype(np.int64)
    deg_tot = in_deg + 1  # self loop
    dinv = 1.0 / np.sqrt(deg_tot.astype(np.float64))

    # slots: real edges only (self loops are added via the xown slab)
    s_src = src
    s_dst = dst

    core = s_dst // DST_PER_CORE
    loc = s_dst % DST_PER_CORE
    blk = loc >> 7
    dloc = loc & 127
    half = (s_src >= HALF).astype(np.int64)
    rowid = s_src - HALF * half

    # group = (core, blk, half); within a group order slots by source row
    # so the gather's HBM reads are ascending (row locality)
    g = (core * NBLK + blk) * 2 + half
    order = np.lexsort((s_src, g))
    g_sorted = g[order]
    cnt = np.bincount(g_sorted, minlength=NCORE * NBLK * 2)
    # static schedule: cross-core max real count per (blk, half) group,
    # 64-quantized (keeps the distinct gather-count register constants few)
    cnt2 = cnt.reshape(NCORE, NBLK * 2)
    maxcnt = ((np.maximum(64, cnt2.max(axis=0)) + 63) // 64) * 64  # [98]
    if maxcnt.max() > MAXCNT_CAP:
        raise RuntimeError(f"group needs {maxcnt.max()} slots > {MAXCNT_CAP}")
    ntile = -(-maxcnt // P)
    col_start = np.concatenate([[0], np.cumsum(ntile)]).astype(np.int64)
    tot_col = int(col_start[-1])
    tot_slot = tot_col * P

    starts = np.zeros_like(cnt)
    starts[1:] = np.cumsum(cnt)[:-1]
    pos_in_group = np.arange(len(order)) - starts[g_sorted]

    blk_s = blk[order]
    half_s = half[order]
    g2 = blk_s * 2 + half_s
    col = col_start[g2] + (pos_in_group >> 7)
    p = pos_in_group & 127
    flat = col * P + p  # slot id within core

    core_s = core[order]
    rowid_s = rowid[order]
    dloc_s = dloc[order]

    # per-core slot arrays. Default: ZROW pad (gathered zeros, sel==0);
    # only the first maxcnt[g] slots of each group are ever read.
    colg = np.repeat(np.arange(NBLK * 2), ntile)  # group of each column
    pad_idx = np.where(colg % 2 == 1, ZROW_B, ZROW_A).astype(np.int16)
    idx_arr = np.empty((NCORE, tot_slot), dtype=np.int16)
    idx_arr[:] = np.repeat(pad_idx, P)[None, :]
    dst_arr = np.full((NCORE, tot_slot), -1.0, dtype=np.float16)

    lin = core_s * tot_slot + flat
    idx_arr.reshape(-1)[lin] = rowid_s.astype(np.int16)
    dst_arr.reshape(-1)[lin] = dloc_s.astype(np.float16)

    # idxs: 16-partition wrap replicated 8x -> [128, tot_slot//16]
    idx_wrap = idx_arr.reshape(NCORE, tot_slot // 16, 16).transpose(0, 2, 1)
    idx_rep = np.tile(idx_wrap, (1, 8, 1)).copy()

    # dstloc: [128, tot_col] fp16 with value at [p, col]
    dst_pc = dst_arr.reshape(NCORE, tot_col, P).transpose(0, 2, 1).copy()

    # dinvdst: [NCORE, 128, NBLK]
    dinvdst = np.ones((NCORE, P, NBLK), dtype=np.float32)
    node = np.arange(N, dtype=np.int64)
    nc_ = node // DST_PER_CORE
    nl = node % DST_PER_CORE
    dinvdst[nc_, nl & 127, nl >> 7] = dinv.astype(np.float32)

    # x half tables, prescaled by dinv[src], fp16, zero padded
    xs = (np.asarray(x, dtype=np.float64) * dinv[:, None]).astype(np.float16)
    xa = np.zeros((XROWS, F), dtype=np.float16)
    xa[:HALF] = xs[:HALF]
    xb = np.zeros((XROWS, F), dtype=np.float16)
    xb[: N - HALF] = xs[HALF:]

    # own-dst slab for the self-loop term: [NCORE, 128(dloc), NBLK*F]
    xpad = np.zeros((NCORE * OUT_ROWS, F), dtype=np.float16)
    xpad_v = xpad.reshape(NCORE, OUT_ROWS, F)
    for c in range(NCORE):
        xpad_v[c, :DST_PER_CORE] = xs[c * DST_PER_CORE:(c + 1) * DST_PER_CORE]
    xown = np.ascontiguousarray(
        xpad_v.reshape(NCORE, NBLK, P, F).transpose(0, 2, 1, 3).reshape(NCORE, P, NBLK * F)
    )

    return maxcnt, xa, xb, idx_rep, dst_pc, dinvdst, xown


def kernel(x, edge_index, W_gcn, b_gcn, W_act, b_act):
    from concourse.bass_utils import run_bass_kernel_spmd

    x = np.ascontiguousarray(np.asarray(x, dtype=np.float32))
    maxcnt, xa, xb, idx_rep, dst_pc, dinvdst, xown = _preprocess(x, edge_index)

    wgT = np.ascontiguousarray(np.asarray(W_gcn, dtype=np.float32).T)
    wact = np.ascontiguousarray(np.asarray(W_act, dtype=np.float32))
    bg = np.ascontiguousarray(np.asarray(b_gcn, dtype=np.float32).reshape(HID, 1))
    ba = np.ascontiguousarray(np.asarray(b_act, dtype=np.float32).reshape(1, A))

    nc = _build_program(tuple(int(v) for v in maxcnt))
    in_maps = [
        {
            "xa": xa,
            "xb": xb,
            "idxs": idx_rep[c],
            "dstloc": dst_pc[c],
            "xown": xown[c],
            "dinvd": dinvdst[c],
            "wgT": wgT,
            "wact": wact,
            "bgcn": bg,
            "bact": ba,
        }
        for c in range(NCORE)
    ]
    trace = bool(os.environ.get("GCN_TRACE"))
    res = run_bass_kernel_spmd(nc, in_maps, core_ids=list(range(NCORE)), trace=trace)
    kernel.last_results = res

    out = np.concatenate([res.results[c]["out"][:DST_PER_CORE] for c in range(NCORE)], axis=0)
    return np.ascontiguousarray(out, dtype=np.float32)


# revision 40
# speedup vs baseline: 1.3232x; 1.3232x over previous
"""GCN dialogue manager kernel for 8 trn2 NeuronCores.

Math (reference):
    h   = x @ W_gcn
    deg = in_deg(dst) + 1   (self loops)
    dinv = rsqrt(deg)
    agg[d] = sum_{e:(s->d)} dinv[s]*dinv[d]*h[s] + dinv[d]^2*h[d] + b_gcn
    out = agg @ W_act + b_act

Kernel strategy (dst-sharded, edges partitioned by destination):
    out[d] = dinv[d] * (sum_{slots of d} x~[s]) @ (W_gcn@W_act) + (b_gcn@W_act + b_act)
  with x~[s] = dinv[s]*x[s] prescaled on the host and stored as fp16
  half-tables (256B rows halve gather traffic vs fp32; int16 gather idx
  limit forces the two-table split). Self loops are ordinary slots.
  - 8 cores each own 6250 destination nodes (49 blocks of 128).
  - Edges (incl. self loops) are bucketed by (core, dst-block, src-half) on
    the host into a static per-group schedule; each group gathers exactly
    maxcnt[g] = cross-core max real count (16-quantized) slots, so no
    full-tile zero-row padding is transferred.
  - On device: dma_gather fetches x~ rows per slot (4 SWDGE queues,
    enlarged dynamic-DMA scratch ring to decouple Q7 descriptor gen from
    SDMA drain).
  - A fp16 one-hot matrix sel[slot, dst] = (dstloc[slot]==iota) routes each
    slot to its dst row via TensorE matmuls accumulated transposed in fp32
    PSUM: accT[feat, dst] += xg[slot, feat]^T @ sel[slot, dst]. Per block:
    outp[dst, A] = accT^T @ (W_gcn@W_act), scaled by dinv[dst] (ACT
    per-partition scale), plus broadcast bias, written out.
"""

import os
import sys

for _p in ("/opt/trn_rl_repo",):
    if _p not in sys.path and os.path.isdir(_p):
        sys.path.insert(0, _p)

import numpy as np

# ---- problem constants (hardcoded per spec) ----
N, E, F, HID, A = 50000, 600000, 128, 128, 64
P = 128                      # partitions
NCORE = 8
DST_PER_CORE = 6250          # N / 8
NBLK = 49                    # ceil(6250/128) dst blocks per core
OUT_ROWS = NBLK * P          # 6272 padded out rows per core
HALF = 25152                 # nodes [0,HALF) in table A, [HALF,...) in table B
XROWS = 25216                # rows per half table (HALF + 64 zero pad rows)
ZROW_A = 25152               # a zero row in table A (explicit pad row)
ZROW_B = 25024               # node 50176 -> xb row 25024 (zero: node >= N)
MAXCNT_CAP = 1024            # hard cap: 1024 slots per gather call (ring cap)
CHUNK = 2                    # dst blocks per compute chunk
_CHUNKS = [(c * CHUNK, min(CHUNK, NBLK - c * CHUNK)) for c in range((NBLK + CHUNK - 1) // CHUNK)]

_prog_cache = {}


def _build_program(maxcnt):
    """Build the Bass program shared by all 8 cores.

    maxcnt: tuple of 98 ints — exact slots gathered per (block, half) group
    (cross-core max real count, 16-quantized), group g = blk*2 + half."""
    key = tuple(maxcnt)
    if key in _prog_cache:
        return _prog_cache[key]

    import concourse.bacc as bacc
    import concourse.mybir as mybir
    import concourse.tile as tile
    from concourse.masks import make_identity

    f32 = mybir.dt.float32
    f16 = mybir.dt.float16
    i32 = mybir.dt.int32
    i16 = mybir.dt.int16
    Alu = mybir.AluOpType
    Act = mybir.ActivationFunctionType

    ntile = [(c + P - 1) // P for c in maxcnt]
    col_start = np.concatenate([[0], np.cumsum(ntile)]).astype(int)
    tot_col = int(col_start[-1])
    tot_slot = tot_col * P

    nc = bacc.Bacc(None, num_swdge_queues=4, dynamic_dma_scratch_size=49152)

    xa = nc.dram_tensor("xa", [XROWS, F], f16, kind="ExternalInput")
    xb = nc.dram_tensor("xb", [XROWS, F], f16, kind="ExternalInput")
    idxs = nc.dram_tensor("idxs", [P, tot_slot // 16], i16, kind="ExternalInput")
    dstloc = nc.dram_tensor("dstloc", [P, tot_col], f16, kind="ExternalInput")
    xown = nc.dram_tensor("xown", [P, NBLK * F], f16, kind="ExternalInput")
    dinvd = nc.dram_tensor("dinvd", [P, NBLK], f32, kind="ExternalInput")
    wgT = nc.dram_tensor("wgT", [HID, F], f32, kind="ExternalInput")
    wact = nc.dram_tensor("wact", [HID, A], f32, kind="ExternalInput")
    bgcn = nc.dram_tensor("bgcn", [HID, 1], f32, kind="ExternalInput")
    bact = nc.dram_tensor("bact", [1, A], f32, kind="ExternalInput")
    out = nc.dram_tensor("out", [OUT_ROWS, A], f32, kind="ExternalOutput")

    with tile.TileContext(nc) as tc:
        with (
            tc.tile_pool(name="const", bufs=1) as cpool,
            tc.tile_pool(name="cpsum", bufs=1, space="PSUM") as cpsum,
            tc.tile_pool(name="xg", bufs=3) as xgpool,
            tc.tile_pool(name="sel", bufs=2) as spool,
            tc.tile_pool(name="acc", bufs=4, space="PSUM") as accpool,
            tc.tile_pool(name="outp", bufs=3, space="PSUM") as outppool,
            tc.tile_pool(name="flush", bufs=8) as fpool,
        ):
            # ---- constants / prologue ----
            # idx table loaded in per-chunk slices so the first gathers
            # don't wait on the whole transfer
            idx_sb = cpool.tile([P, tot_slot // 16], i16)
            for (b0, ncb) in _CHUNKS:
                a = int(col_start[b0 * 2]) * P // 16
                z = int(col_start[(b0 + ncb) * 2]) * P // 16
                nc.sync.dma_start(out=idx_sb[:, a:z], in_=idxs[:, a:z])
            dstloc_sb = cpool.tile([P, tot_col], f16)
            nc.sync.dma_start(out=dstloc_sb[:], in_=dstloc[:])

            dinvdst = cpool.tile([P, NBLK], f32)
            nc.sync.dma_start(out=dinvdst[:], in_=dinvd[:])

            iota_i = cpool.tile([P, P], i32)
            nc.gpsimd.iota(iota_i[:], pattern=[[1, P]], base=0, channel_multiplier=0)
            iota_f = cpool.tile([P, P], f16)
            nc.vector.tensor_copy(out=iota_f[:], in_=iota_i[:])

            # own-dst x~ slab (self-loop term) + fp16 identity for its matmul
            xown_sb = cpool.tile([P, NBLK * F], f16)
            nc.sync.dma_start(out=xown_sb[:], in_=xown[:])
            ident_f = cpool.tile([P, P], f32)
            make_identity(nc, ident_f[:])
            ident16 = cpool.tile([P, P], f16)
            nc.vector.tensor_copy(out=ident16[:], in_=ident_f[:])

            wgT_sb = cpool.tile([HID, F], f32)
            nc.sync.dma_start(out=wgT_sb[:], in_=wgT[:])
            wact_sb = cpool.tile([HID, A], f32)
            nc.sync.dma_start(out=wact_sb[:], in_=wact[:])
            wf_ps = cpsum.tile([F, A], f32, space="PSUM", tag="cps")
            nc.tensor.matmul(wf_ps[:], lhsT=wgT_sb[:], rhs=wact_sb[:], start=True, stop=True)
            wf_sb = cpool.tile([F, A], f32)
            nc.vector.tensor_copy(out=wf_sb[:], in_=wf_ps[:])

            bgcn_sb = cpool.tile([HID, 1], f32)
            nc.sync.dma_start(out=bgcn_sb[:], in_=bgcn[:])
            bact_sb = cpool.tile([1, A], f32)
            nc.sync.dma_start(out=bact_sb[:], in_=bact[:])
            cb_ps = cpsum.tile([1, A], f32, space="PSUM", tag="cps")
            nc.tensor.matmul(cb_ps[:], lhsT=bgcn_sb[:], rhs=wact_sb[:], start=True, stop=True)
            cb_sb = cpool.tile([1, A], f32)
            nc.vector.tensor_copy(out=cb_sb[:], in_=cb_ps[:])
            nc.vector.tensor_tensor(out=cb_sb[:], in0=cb_sb[:], in1=bact_sb[:], op=Alu.add)
            bias_full = cpool.tile([P, A], f32)
            nc.gpsimd.partition_broadcast(bias_full[:], cb_sb[:])

            num_regs = {v: nc.gpsimd.to_reg(v)
                        for v in sorted(set(int(c) for c in maxcnt))}

            # prime the gather buffers: the tail of each group's last tile is
            # never written by the gather; sel is 0 there, but stale SBUF bits
            # could be NaN and 0*NaN would poison the accumulation.
            max_ncols = max(
                int(col_start[(b0 + ncb) * 2] - col_start[b0 * 2])
                for (b0, ncb) in _CHUNKS
            )
            for _ in range(3):
                zt = xgpool.tile([P, max_ncols, F], f16, tag="xg")
                nc.vector.memset(zt[:], 0.0)

            # ---- main loop over chunks of dst blocks ----
            qn = 0
            for ci, (b0, ncb) in enumerate(_CHUNKS):
                c0 = int(col_start[b0 * 2])
                ncols = int(col_start[(b0 + ncb) * 2] - c0)
                xg = xgpool.tile([P, max_ncols, F], f16, tag="xg")
                for i in range(ncb):
                    for h, tab in ((0, xa), (1, xb)):
                        g = (b0 + i) * 2 + h
                        num = int(maxcnt[g])
                        nt = int(ntile[g])
                        crel = int(col_start[g]) - c0
                        s0 = int(col_start[g]) * P
                        nc.gpsimd.dma_gather(
                            xg[:, crel: crel + nt, :],
                            tab[:],
                            idx_sb[:, s0 // 16: s0 // 16 + (num + 15) // 16],
                            num,
                            num_regs[num],
                            F,
                            queue_num=qn % 4,
                        )
                        qn += 1
                qn += 1  # rotate the queue each chunk starts on
                # fp16 one-hot selection (broadcast TT on DVE)
                sel = spool.tile([P, ncols, P], f16, tag="sel")
                nc.vector.tensor_tensor(
                    out=sel[:],
                    in0=dstloc_sb[:, c0:c0 + ncols].unsqueeze(2).broadcast_to([P, ncols, P]),
                    in1=iota_f[:].unsqueeze(1).broadcast_to([P, ncols, P]),
                    op=Alu.is_equal,
                )
                for i in range(ncb):
                    b = b0 + i
                    gA, gB = b * 2, b * 2 + 1
                    accT = accpool.tile([P, P], f32, space="PSUM", tag="acc")
                    # self-loop term: accT[feat, dst] += xown_b[dst, feat]^T
                    nc.tensor.matmul(
                        accT[:],
                        lhsT=xown_sb[:, b * F:(b + 1) * F],
                        rhs=ident16[:],
                        start=True,
                        stop=False,
                    )
                    cols = list(range(int(col_start[gA]) - c0, int(col_start[gB + 1]) - c0))
                    for j, col in enumerate(cols):
                        nc.tensor.matmul(
                            accT[:],
                            lhsT=xg[:, col, :],
                            rhs=sel[:, col, :],
                            start=False,
                            stop=(j == len(cols) - 1),
                        )
                    # flush block b: accT[feat, dst] -> out rows
                    accS = fpool.tile([P, P], f32, tag="accS")
                    nc.scalar.activation(accS[:], accT[:], Act.Copy)
                    outp = outppool.tile([P, A], f32, space="PSUM", tag="outp")
                    nc.tensor.matmul(outp[:], lhsT=accS[:], rhs=wf_sb[:], start=True, stop=True)
                    out_sb = fpool.tile([P, A], f32, tag="outs")
                    nc.scalar.activation(out_sb[:], outp[:], Act.Copy, scale=dinvdst[:, b:b + 1])
                    nc.vector.tensor_tensor(out=out_sb[:], in0=out_sb[:], in1=bias_full[:], op=Alu.add)
                    nc.sync.dma_start(out=out[b * P:(b + 1) * P, :], in_=out_sb[:])

    nc.compile()
    _prog_cache[key] = nc
    return nc


def _preprocess(x, edge_index):
    """Host-side sharding: bucket edges by (core, dst block, src half), build
    the per-slot gather index/dst-position arrays, and prescale x rows by
    dinv[src] into fp16 half tables."""
    src = np.asarray(edge_index[0], dtype=np.int64)
    dst = np.asarray(edge_index[1], dtype=np.int64)

    in_deg = np.bincount(dst, minlength=N).astype(np.int64)
    deg_tot = in_deg + 1  # self loop
    dinv = 1.0 / np.sqrt(deg_tot.astype(np.float64))

    # slots: real edges only (self loops are added via the xown slab)
    s_src = src
    s_dst = dst

    core = s_dst // DST_PER_CORE
    loc = s_dst % DST_PER_CORE
    blk = loc >> 7
    dloc = loc & 127
    half = (s_src >= HALF).astype(np.int64)
    rowid = s_src - HALF * half

    # group = (core, blk, half); within a group order slots by source row
    # so the gather's HBM reads are ascending (row locality)
    g = (core * NBLK + blk) * 2 + half
    order = np.lexsort((s_src, g))
    g_sorted = g[order]
    cnt = np.bincount(g_sorted, minlength=NCORE * NBLK * 2)
    # static schedule: cross-core max real count per (blk, half) group,
    # 64-quantized (keeps the distinct gather-count register constants few)
    cnt2 = cnt.reshape(NCORE, NBLK * 2)
    maxcnt = ((np.maximum(64, cnt2.max(axis=0)) + 63) // 64) * 64  # [98]
    if maxcnt.max() > MAXCNT_CAP:
        raise RuntimeError(f"group needs {maxcnt.max()} slots > {MAXCNT_CAP}")
    ntile = -(-maxcnt // P)
    col_start = np.concatenate([[0], np.cumsum(ntile)]).astype(np.int64)
    tot_col = int(col_start[-1])
    tot_slot = tot_col * P

    starts = np.zeros_like(cnt)
    starts[1:] = np.cumsum(cnt)[:-1]
    pos_in_group = np.arange(len(order)) - starts[g_sorted]

    blk_s = blk[order]
    half_s = half[order]
    g2 = blk_s * 2 + half_s
    col = col_start[g2] + (pos_in_group >> 7)
    p = pos_in_group & 127
    flat = col * P + p  # slot id within core

    core_s = core[order]
    rowid_s = rowid[order]
    dloc_s = dloc[order]

    # per-core slot arrays. Default: ZROW pad (gathered zeros, sel==0);
    # only the first maxcnt[g] slots of each group are ever read.
    colg = np.repeat(np.arange(NBLK * 2), ntile)  # group of each column
    pad_idx = np.where(colg % 2 == 1, ZROW_B, ZROW_A).astype(np.int16)
    idx_arr = np.empty((NCORE, tot_slot), dtype=np.int16)
    idx_arr[:] = np.repeat(pad_idx, P)[None, :]
    dst_arr = np.full((NCORE, tot_slot), -1.0, dtype=np.float16)

    lin = core_s * tot_slot + flat
    idx_arr.reshape(-1)[lin] = rowid_s.astype(np.int16)
    dst_arr.reshape(-1)[lin] = dloc_s.astype(np.float16)

    # idxs: 16-partition wrap replicated 8x -> [128, tot_slot//16]
    idx_wrap = idx_arr.reshape(NCORE, tot_slot // 16, 16).transpose(0, 2, 1)
    idx_rep = np.tile(idx_wrap, (1, 8, 1)).copy()

    # dstloc: [128, tot_col] fp16 with value at [p, col]
    dst_pc = dst_arr.reshape(NCORE, tot_col, P).transpose(0, 2, 1).copy()

    # dinvdst: [NCORE, 128, NBLK]
    dinvdst = np.ones((NCORE, P, NBLK), dtype=np.float32)
    node = np.arange(N, dtype=np.int64)
    nc_ = node // DST_PER_CORE
    nl = node % DST_PER_CORE
    dinvdst[nc_, nl & 127, nl >> 7] = dinv.astype(np.float32)

    # x half tables, prescaled by dinv[src], fp16, zero padded
    xs = (np.asarray(x, dtype=np.float64) * dinv[:, None]).astype(np.float16)
    xa = np.zeros((XROWS, F), dtype=np.float16)
    xa[:HALF] = xs[:HALF]
    xb = np.zeros((XROWS, F), dtype=np.float16)
    xb[: N - HALF] = xs[HALF:]

    # own-dst slab for the self-loop term: [NCORE, 128(dloc), NBLK*F]
    xpad = np.zeros((NCORE * OUT_ROWS, F), dtype=np.float16)
    xpad_v = xpad.reshape(NCORE, OUT_ROWS, F)
    for c in range(NCORE):
        xpad_v[c, :DST_PER_CORE] = xs[c * DST_PER_CORE:(c + 1) * DST_PER_CORE]
    xown = np.ascontiguousarray(
        xpad_v.reshape(NCORE, NBLK, P, F).transpose(0, 2, 1, 3).reshape(NCORE, P, NBLK * F)
    )

    return maxcnt, xa, xb, idx_rep, dst_pc, dinvdst, xown


def kernel(x, edge_index, W_gcn, b_gcn, W_act, b_act):
    from concourse.bass_utils import run_bass_kernel_spmd

    x = np.ascontiguousarray(np.asarray(x, dtype=np.float32))
    maxcnt, xa, xb, idx_rep, dst_pc, dinvdst, xown = _preprocess(x, edge_index)

    wgT = np.ascontiguousarray(np.asarray(W_gcn, dtype=np.float32).T)
    wact = np.ascontiguousarray(np.asarray(W_act, dtype=np.float32))
    bg = np.ascontiguousarray(np.asarray(b_gcn, dtype=np.float32).reshape(HID, 1))
    ba = np.ascontiguousarray(np.asarray(b_act, dtype=np.float32).reshape(1, A))

    nc = _build_program(tuple(int(v) for v in maxcnt))
    in_maps = [
        {
            "xa": xa,
            "xb": xb,
            "idxs": idx_rep[c],
            "dstloc": dst_pc[c],
            "xown": xown[c],
            "dinvd": dinvdst[c],
            "wgT": wgT,
            "wact": wact,
            "bgcn": bg,
            "bact": ba,
        }
        for c in range(NCORE)
    ]
    trace = bool(os.environ.get("GCN_TRACE"))
    res = run_bass_kernel_spmd(nc, in_maps, core_ids=list(range(NCORE)), trace=trace)
    kernel.last_results = res

    out = np.concatenate([res.results[c]["out"][:DST_PER_CORE] for c in range(NCORE)], axis=0)
    return np.ascontiguousarray(out, dtype=np.float32)


# revision 41
# speedup vs baseline: 1.3346x; 1.0086x over previous
"""GCN dialogue manager kernel for 8 trn2 NeuronCores.

Math (reference):
    h   = x @ W_gcn
    deg = in_deg(dst) + 1   (self loops)
    dinv = rsqrt(deg)
    agg[d] = sum_{e:(s->d)} dinv[s]*dinv[d]*h[s] + dinv[d]^2*h[d] + b_gcn
    out = agg @ W_act + b_act

Kernel strategy (dst-sharded, edges partitioned by destination):
    out[d] = dinv[d] * (sum_{slots of d} x~[s]) @ (W_gcn@W_act) + (b_gcn@W_act + b_act)
  with x~[s] = dinv[s]*x[s] prescaled on the host and stored as fp16
  half-tables (256B rows halve gather traffic vs fp32; int16 gather idx
  limit forces the two-table split). Self loops are ordinary slots.
  - 8 cores each own 6250 destination nodes (49 blocks of 128).
  - Edges (incl. self loops) are bucketed by (core, dst-block, src-half) on
    the host into a static per-group schedule; each group gathers exactly
    maxcnt[g] = cross-core max real count (16-quantized) slots, so no
    full-tile zero-row padding is transferred.
  - On device: dma_gather fetches x~ rows per slot (4 SWDGE queues,
    enlarged dynamic-DMA scratch ring to decouple Q7 descriptor gen from
    SDMA drain).
  - A fp16 one-hot matrix sel[slot, dst] = (dstloc[slot]==iota) routes each
    slot to its dst row via TensorE matmuls accumulated transposed in fp32
    PSUM: accT[feat, dst] += xg[slot, feat]^T @ sel[slot, dst]. Per block:
    outp[dst, A] = accT^T @ (W_gcn@W_act), scaled by dinv[dst] (ACT
    per-partition scale), plus broadcast bias, written out.
"""

import os
import sys

for _p in ("/opt/trn_rl_repo",):
    if _p not in sys.path and os.path.isdir(_p):
        sys.path.insert(0, _p)

import numpy as np

# ---- problem constants (hardcoded per spec) ----
N, E, F, HID, A = 50000, 600000, 128, 128, 64
P = 128                      # partitions
NCORE = 8
DST_PER_CORE = 6250          # N / 8
NBLK = 49                    # ceil(6250/128) dst blocks per core
OUT_ROWS = NBLK * P          # 6272 padded out rows per core
HALF = 25152                 # nodes [0,HALF) in table A, [HALF,...) in table B
XROWS = 25216                # rows per half table (HALF + 64 zero pad rows)
ZROW_A = 25152               # a zero row in table A (explicit pad row)
ZROW_B = 25024               # node 50176 -> xb row 25024 (zero: node >= N)
MAXCNT_CAP = 1024            # hard cap: 1024 slots per gather call (ring cap)
CHUNK = 2                    # dst blocks per compute chunk
_CHUNKS = [(c * CHUNK, min(CHUNK, NBLK - c * CHUNK)) for c in range((NBLK + CHUNK - 1) // CHUNK)]

_prog_cache = {}


def _build_program(maxcnt):
    """Build the Bass program shared by all 8 cores.

    maxcnt: tuple of 98 ints — exact slots gathered per (block, half) group
    (cross-core max real count, 16-quantized), group g = blk*2 + half."""
    key = tuple(maxcnt)
    if key in _prog_cache:
        return _prog_cache[key]

    import concourse.bacc as bacc
    import concourse.mybir as mybir
    import concourse.tile as tile
    from concourse.masks import make_identity

    f32 = mybir.dt.float32
    f16 = mybir.dt.float16
    i32 = mybir.dt.int32
    i16 = mybir.dt.int16
    Alu = mybir.AluOpType
    Act = mybir.ActivationFunctionType

    ntile = [(c + P - 1) // P for c in maxcnt]
    col_start = np.concatenate([[0], np.cumsum(ntile)]).astype(int)
    tot_col = int(col_start[-1])
    tot_slot = tot_col * P

    nc = bacc.Bacc(None, num_swdge_queues=4, dynamic_dma_scratch_size=49152)

    xa = nc.dram_tensor("xa", [XROWS, F], f16, kind="ExternalInput")
    xb = nc.dram_tensor("xb", [XROWS, F], f16, kind="ExternalInput")
    idxs = nc.dram_tensor("idxs", [P, tot_slot // 16], i16, kind="ExternalInput")
    dstloc = nc.dram_tensor("dstloc", [P, tot_col], f16, kind="ExternalInput")
    xown = nc.dram_tensor("xown", [P, NBLK * F], f16, kind="ExternalInput")
    dinvd = nc.dram_tensor("dinvd", [P, NBLK], f32, kind="ExternalInput")
    wgT = nc.dram_tensor("wgT", [HID, F], f32, kind="ExternalInput")
    wact = nc.dram_tensor("wact", [HID, A], f32, kind="ExternalInput")
    bgcn = nc.dram_tensor("bgcn", [HID, 1], f32, kind="ExternalInput")
    bact = nc.dram_tensor("bact", [1, A], f32, kind="ExternalInput")
    out = nc.dram_tensor("out", [OUT_ROWS, A], f32, kind="ExternalOutput")

    with tile.TileContext(nc) as tc:
        with (
            tc.tile_pool(name="const", bufs=1) as cpool,
            tc.tile_pool(name="cpsum", bufs=1, space="PSUM") as cpsum,
            tc.tile_pool(name="xg", bufs=3) as xgpool,
            tc.tile_pool(name="sel", bufs=2) as spool,
            tc.tile_pool(name="acc", bufs=4, space="PSUM") as accpool,
            tc.tile_pool(name="outp", bufs=3, space="PSUM") as outppool,
            tc.tile_pool(name="flush", bufs=6) as fpool,
        ):
            # ---- constants / prologue ----
            # idx table loaded in per-chunk slices so the first gathers
            # don't wait on the whole transfer
            idx_sb = cpool.tile([P, tot_slot // 16], i16)
            for (b0, ncb) in _CHUNKS:
                a = int(col_start[b0 * 2]) * P // 16
                z = int(col_start[(b0 + ncb) * 2]) * P // 16
                nc.sync.dma_start(out=idx_sb[:, a:z], in_=idxs[:, a:z])
            dstloc_sb = cpool.tile([P, tot_col], f16)
            nc.sync.dma_start(out=dstloc_sb[:], in_=dstloc[:])

            dinvdst = cpool.tile([P, NBLK], f32)
            nc.sync.dma_start(out=dinvdst[:], in_=dinvd[:])

            iota_i = cpool.tile([P, P], i32)
            nc.gpsimd.iota(iota_i[:], pattern=[[1, P]], base=0, channel_multiplier=0)
            iota_f = cpool.tile([P, P], f16)
            nc.vector.tensor_copy(out=iota_f[:], in_=iota_i[:])

            # own-dst x~ slab (self-loop term) + fp16 identity for its matmul
            xown_sb = cpool.tile([P, NBLK * F], f16)
            nc.sync.dma_start(out=xown_sb[:], in_=xown[:])
            ident_f = cpool.tile([P, P], f32)
            make_identity(nc, ident_f[:])
            ident16 = cpool.tile([P, P], f16)
            nc.vector.tensor_copy(out=ident16[:], in_=ident_f[:])

            wgT_sb = cpool.tile([HID, F], f32)
            nc.sync.dma_start(out=wgT_sb[:], in_=wgT[:])
            wact_sb = cpool.tile([HID, A], f32)
            nc.sync.dma_start(out=wact_sb[:], in_=wact[:])
            wf_ps = cpsum.tile([F, A], f32, space="PSUM", tag="cps")
            nc.tensor.matmul(wf_ps[:], lhsT=wgT_sb[:], rhs=wact_sb[:], start=True, stop=True)
            wf_sb = cpool.tile([F, A], f32)
            nc.vector.tensor_copy(out=wf_sb[:], in_=wf_ps[:])

            bgcn_sb = cpool.tile([HID, 1], f32)
            nc.sync.dma_start(out=bgcn_sb[:], in_=bgcn[:])
            bact_sb = cpool.tile([1, A], f32)
            nc.sync.dma_start(out=bact_sb[:], in_=bact[:])
            cb_ps = cpsum.tile([1, A], f32, space="PSUM", tag="cps")
            nc.tensor.matmul(cb_ps[:], lhsT=bgcn_sb[:], rhs=wact_sb[:], start=True, stop=True)
            cb_sb = cpool.tile([1, A], f32)
            nc.vector.tensor_copy(out=cb_sb[:], in_=cb_ps[:])
            nc.vector.tensor_tensor(out=cb_sb[:], in0=cb_sb[:], in1=bact_sb[:], op=Alu.add)
            bias_full = cpool.tile([P, A], f32)
            nc.gpsimd.partition_broadcast(bias_full[:], cb_sb[:])

            num_regs = {v: nc.gpsimd.to_reg(v)
                        for v in sorted(set(int(c) for c in maxcnt))}

            # prime the gather buffers: the tail of each group's last tile is
            # never written by the gather; sel is 0 there, but stale SBUF bits
            # could be NaN and 0*NaN would poison the accumulation.
            max_ncols = max(
                int(col_start[(b0 + ncb) * 2] - col_start[b0 * 2])
                for (b0, ncb) in _CHUNKS
            )
            for _ in range(3):
                zt = xgpool.tile([P, max_ncols, F], f16, tag="xg")
                nc.vector.memset(zt[:], 0.0)

            # ---- main loop over chunks of dst blocks ----
            qn = 0
            for ci, (b0, ncb) in enumerate(_CHUNKS):
                c0 = int(col_start[b0 * 2])
                ncols = int(col_start[(b0 + ncb) * 2] - c0)
                xg = xgpool.tile([P, max_ncols, F], f16, tag="xg")
                for i in range(ncb):
                    for h, tab in ((0, xa), (1, xb)):
                        g = (b0 + i) * 2 + h
                        num = int(maxcnt[g])
                        nt = int(ntile[g])
                        crel = int(col_start[g]) - c0
                        s0 = int(col_start[g]) * P
                        nc.gpsimd.dma_gather(
                            xg[:, crel: crel + nt, :],
                            tab[:],
                            idx_sb[:, s0 // 16: s0 // 16 + (num + 15) // 16],
                            num,
                            num_regs[num],
                            F,
                            queue_num=qn % 4,
                        )
                        qn += 1
                qn += 1  # rotate the queue each chunk starts on
                # fp16 one-hot selection (broadcast TT on DVE)
                sel = spool.tile([P, ncols, P], f16, tag="sel")
                nc.vector.tensor_tensor(
                    out=sel[:],
                    in0=dstloc_sb[:, c0:c0 + ncols].unsqueeze(2).broadcast_to([P, ncols, P]),
                    in1=iota_f[:].unsqueeze(1).broadcast_to([P, ncols, P]),
                    op=Alu.is_equal,
                )
                for i in range(ncb):
                    b = b0 + i
                    gA, gB = b * 2, b * 2 + 1
                    accT = accpool.tile([P, P], f32, space="PSUM", tag="acc")
                    # self-loop term: accT[feat, dst] += xown_b[dst, feat]^T
                    nc.tensor.matmul(
                        accT[:],
                        lhsT=xown_sb[:, b * F:(b + 1) * F],
                        rhs=ident16[:],
                        start=True,
                        stop=False,
                    )
                    cols = list(range(int(col_start[gA]) - c0, int(col_start[gB + 1]) - c0))
                    for j, col in enumerate(cols):
                        nc.tensor.matmul(
                            accT[:],
                            lhsT=xg[:, col, :],
                            rhs=sel[:, col, :],
                            start=False,
                            stop=(j == len(cols) - 1),
                        )
                    # flush block b: accT[feat, dst] -> out rows
                    accS = fpool.tile([P, P], f32, tag="accS")
                    nc.scalar.activation(accS[:], accT[:], Act.Copy)
                    outp = outppool.tile([P, A], f32, space="PSUM", tag="outp")
                    nc.tensor.matmul(outp[:], lhsT=accS[:], rhs=wf_sb[:], start=True, stop=True)
                    out_sb = fpool.tile([P, A], f32, tag="outs")
                    nc.scalar.activation(out_sb[:], outp[:], Act.Copy, scale=dinvdst[:, b:b + 1])
                    nc.vector.tensor_tensor(out=out_sb[:], in0=out_sb[:], in1=bias_full[:], op=Alu.add)
                    nc.sync.dma_start(out=out[b * P:(b + 1) * P, :], in_=out_sb[:])

    nc.compile()
    _prog_cache[key] = nc
    return nc


def _preprocess(x, edge_index):
    """Host-side sharding: bucket edges by (core, dst block, src half), build
    the per-slot gather index/dst-position arrays, and prescale x rows by
    dinv[src] into fp16 half tables."""
    src = np.asarray(edge_index[0], dtype=np.int64)
    dst = np.asarray(edge_index[1], dtype=np.int64)

    in_deg = np.bincount(dst, minlength=N).astype(np.int64)
    deg_tot = in_deg + 1  # self loop
    dinv = 1.0 / np.sqrt(deg_tot.astype(np.float64))

    # slots: real edges only (self loops are added via the xown slab)
    s_src = src
    s_dst = dst

    core = s_dst // DST_PER_CORE
    loc = s_dst % DST_PER_CORE
    blk = loc >> 7
    dloc = loc & 127
    half = (s_src >= HALF).astype(np.int64)
    rowid = s_src - HALF * half

    # group = (core, blk, half); within a group order slots by source row
    # so the gather's HBM reads are ascending (row locality)
    g = (core * NBLK + blk) * 2 + half
    order = np.lexsort((s_src, g))
    g_sorted = g[order]
    cnt = np.bincount(g_sorted, minlength=NCORE * NBLK * 2)
    # static schedule: cross-core max real count per (blk, half) group,
    # 64-quantized (keeps the distinct gather-count register constants few)
    cnt2 = cnt.reshape(NCORE, NBLK * 2)
    maxcnt = ((np.maximum(64, cnt2.max(axis=0)) + 63) // 64) * 64  # [98]
    if maxcnt.max() > MAXCNT_CAP:
        raise RuntimeError(f"group needs {maxcnt.max()} slots > {MAXCNT_CAP}")
    ntile = -(-maxcnt // P)
    col_start = np.concatenate([[0], np.cumsum(ntile)]).astype(np.int64)
    tot_col = int(col_start[-1])
    tot_slot = tot_col * P

    starts = np.zeros_like(cnt)
    starts[1:] = np.cumsum(cnt)[:-1]
    pos_in_group = np.arange(len(order)) - starts[g_sorted]

    blk_s = blk[order]
    half_s = half[order]
    g2 = blk_s * 2 + half_s
    col = col_start[g2] + (pos_in_group >> 7)
    p = pos_in_group & 127
    flat = col * P + p  # slot id within core

    core_s = core[order]
    rowid_s = rowid[order]
    dloc_s = dloc[order]

    # per-core slot arrays. Default: ZROW pad (gathered zeros, sel==0);
    # only the first maxcnt[g] slots of each group are ever read.
    colg = np.repeat(np.arange(NBLK * 2), ntile)  # group of each column
    pad_idx = np.where(colg % 2 == 1, ZROW_B, ZROW_A).astype(np.int16)
    idx_arr = np.empty((NCORE, tot_slot), dtype=np.int16)
    idx_arr[:] = np.repeat(pad_idx, P)[None, :]
    dst_arr = np.full((NCORE, tot_slot), -1.0, dtype=np.float16)

    lin = core_s * tot_slot + flat
    idx_arr.reshape(-1)[lin] = rowid_s.astype(np.int16)
    dst_arr.reshape(-1)[lin] = dloc_s.astype(np.float16)

    # idxs: 16-partition wrap replicated 8x -> [128, tot_slot//16]
    idx_wrap = idx_arr.reshape(NCORE, tot_slot // 16, 16).transpose(0, 2, 1)
    idx_rep = np.tile(idx_wrap, (1, 8, 1)).copy()

    # dstloc: [128, tot_col] fp16 with value at [p, col]
    dst_pc = dst_arr.reshape(NCORE, tot_col, P).transpose(0, 2, 1).copy()

    # dinvdst: [NCORE, 128, NBLK]
    dinvdst = np.ones((NCORE, P, NBLK), dtype=np.float32)
    node = np.arange(N, dtype=np.int64)
    nc_ = node // DST_PER_CORE
    nl = node % DST_PER_CORE
    dinvdst[nc_, nl & 127, nl >> 7] = dinv.astype(np.float32)

    # x half tables, prescaled by dinv[src], fp16, zero padded
    xs = (np.asarray(x, dtype=np.float64) * dinv[:, None]).astype(np.float16)
    xa = np.zeros((XROWS, F), dtype=np.float16)
    xa[:HALF] = xs[:HALF]
    xb = np.zeros((XROWS, F), dtype=np.float16)
    xb[: N - HALF] = xs[HALF:]

    # own-dst slab for the self-loop term: [NCORE, 128(dloc), NBLK*F]
    xpad = np.zeros((NCORE * OUT_ROWS, F), dtype=np.float16)
    xpad_v = xpad.reshape(NCORE, OUT_ROWS, F)
    for c in range(NCORE):
        xpad_v[c, :DST_PER_CORE] = xs[c * DST_PER_CORE:(c + 1) * DST_PER_CORE]
    xown = np.ascontiguousarray(
        xpad_v.reshape(NCORE, NBLK, P, F).transpose(0, 2, 1, 3).reshape(NCORE, P, NBLK * F)
    )

    return maxcnt, xa, xb, idx_rep, dst_pc, dinvdst, xown


def kernel(x, edge_index, W_gcn, b_gcn, W_act, b_act):
    from concourse.bass_utils import run_bass_kernel_spmd

    x = np.ascontiguousarray(np.asarray(x, dtype=np.float32))
    maxcnt, xa, xb, idx_rep, dst_pc, dinvdst, xown = _preprocess(x, edge_index)

    wgT = np.ascontiguousarray(np.asarray(W_gcn, dtype=np.float32).T)
    wact = np.ascontiguousarray(np.asarray(W_act, dtype=np.float32))
    bg = np.ascontiguousarray(np.asarray(b_gcn, dtype=np.float32).reshape(HID, 1))
    ba = np.ascontiguousarray(np.asarray(b_act, dtype=np.float32).reshape(1, A))

    nc = _build_program(tuple(int(v) for v in maxcnt))
    in_maps = [
        {
            "xa": xa,
            "xb": xb,
            "idxs": idx_rep[c],
            "dstloc": dst_pc[c],
            "xown": xown[c],
            "dinvd": dinvdst[c],
            "wgT": wgT,
            "wact": wact,
            "bgcn": bg,
            "bact": ba,
        }
        for c in range(NCORE)
    ]
    trace = bool(os.environ.get("GCN_TRACE"))
    res = run_bass_kernel_spmd(nc, in_maps, core_ids=list(range(NCORE)), trace=trace)
    kernel.last_results = res

    out = np.concatenate([res.results[c]["out"][:DST_PER_CORE] for c in range(NCORE)], axis=0)
    return np.ascontiguousarray(out, dtype=np.float32)
